# revision 21
# baseline (speedup 1.0000x reference)
"""Trainium2 Bass kernel for nn_Attention_47553877901998.

GQA attention block: rmsnorm -> q/kv proj -> per-head l2norm*(gamma+1)*sqrt(dh)
-> softcapped causal attention (summing over the 2-query-head group) -> out proj.

Sharding over 8 cores: core c owns batch b = c//4 and kv-heads {2*(c%4), 2*(c%4)+1}
(4 query heads). Each core emits a partial [2048, 1024] output for its batch;
the host sums the 4 partials per batch.

Device-side math notes:
  * norm_w is folded into the projection weights on the host; the rmsnorm row
    scale rs[i] cancels inside the q/k l2norms, so only v is scaled by rs.
  * the q-side l2norm gammas and the k-side gammas are COMBINED into one
    per-(head,dh) vector gvec = (q_gamma+1)(k_gamma+1)*DH applied to q only;
    the k rows stay raw and the key 1/||k|| norm (x0.02 softcap prescale) is
    folded into the Tanh activation's per-partition scale operand.
  * softcap bounds logits to +-6.25 after the dh**-0.5 scale, so softmax runs
    without max-subtraction; a 0-fill affine_select after exp handles the
    causal mask.
  * ACT (scalar) engine is the critical resource: it runs tanh+exp plus only
    light B-phase work. Schedule: stage 1 projects token tiles 0..7; stage 2
    runs every instance's hf=0 attention (keys/queries 0:1024) interleaved
    with the projection of tiles 8..15; stage 3 runs the hf=1 halves with
    the first half of the output projection woven into the jt loops.
  * the walrus build here encodes at most one sem-wait per instruction and
    rejects custom-DVE/TensorTensorReduce ISA structs, so only stock BIR ops
    are used and _split_waits() hoists Tile's extra waits onto NOPs.
"""

import os
import sys

import numpy as np
import ml_dtypes

for _p in ("/root/.axon_site/_ro/trn_rl_repo", "/opt/trn_rl_repo"):
    if os.path.isdir(_p) and _p not in sys.path:
        sys.path.insert(0, _p)

import concourse.bass as bass
import concourse.mybir as mybir
import concourse.tile as tile
from concourse.bass import ds, ts
from concourse.bass_utils import run_bass_kernel_spmd
from concourse.masks import make_identity

F32 = mybir.dt.float32
BF16 = mybir.dt.bfloat16
AF = mybir.ActivationFunctionType
ALU = mybir.AluOpType

B, N, D = 2, 2048, 1024
H, QH, DH = 8, 16, 64
P = 128
NT = N // P              # 16 row tiles
KT = D // P              # 8 contraction tiles
HN = N // 2
EPS = float(np.finfo(np.float32).eps)


def _split_waits(nc):
    """Hoist all-but-one sync wait per instruction into preceding NOPs.

    The walrus build in this container encodes at most ONE sem-wait per
    instruction ("Too many sync wait commands"); Tile's scheduler attaches
    several. A single-wait NOP on the same engine immediately before the
    instruction preserves the happens-before ordering exactly.
    """
    import bass_rust as _br
    n = 0
    for blk in nc.m.functions[0].blocks:
        out = []
        for ins in blk.instructions:
            si = ins.sync_info
            if si is not None and si.on_wait and len(si.on_wait) > 1:
                waits = list(si.on_wait)
                eng = ins.engine
                for w in waits[:-1]:
                    n += 1
                    out.append(mybir.InstNoOp(
                        name=f"waitsplit-{n}",
                        engine=eng,
                        ins=[], outs=[],
                        sync_info=_br.SyncInfo(on_wait=[w], on_update=[]),
                    ))
                si.on_wait = [waits[-1]]
            out.append(ins)
        blk.instructions = out
    return n


def build_nc(split_waits=True):
    nc = bass.Bass("TRN2")

    tok_d = nc.dram_tensor("tok", [N, D], BF16, kind="ExternalInput")
    wqkv_d = nc.dram_tensor("wqkv", [D, 512], BF16, kind="ExternalInput")
    wout_d = nc.dram_tensor("wout", [2, P, D], BF16, kind="ExternalInput")
    gv_d = nc.dram_tensor("gvec", [256], BF16, kind="ExternalInput")
    out_d = nc.dram_tensor("out_p", [N, D], BF16, kind="ExternalOutput")

    with tile.TileContext(nc) as tc:
        with (
            tc.tile_pool(name="const", bufs=1) as const,
            tc.tile_pool(name="big", bufs=1) as big,
            tc.tile_pool(name="work", bufs=3) as work,
            tc.tile_pool(name="att", bufs=2) as att,
            tc.tile_pool(name="nrm", bufs=2) as nrm,
            tc.tile_pool(name="drp", bufs=4, space="DRAM") as drp,
            tc.tile_pool(name="pps", bufs=2, space="PSUM") as pps,
            tc.tile_pool(name="pst", bufs=2, space="PSUM") as pst,
            tc.tile_pool(name="pot", bufs=2, space="PSUM") as pot,
        ):
            # ---- constants / weights ----
            ident = const.tile([P, P], BF16)
            make_identity(nc, ident)
            ones_t = const.tile([65, 64], BF16)
            nc.vector.memset(ones_t, 1.0)
            wqkv_sb = const.tile([P, KT, 512], BF16)
            nc.sync.dma_start(out=wqkv_sb,
                              in_=wqkv_d.rearrange("(k p) n -> p k n", p=P))
            gv_ap = gv_d[:]
            gvec_sb = const.tile([P, 256], BF16)
            nc.sync.dma_start(
                out=gvec_sb,
                in_=bass.AP(tensor=gv_ap.tensor, offset=gv_ap.offset,
                            ap=[[0, P], [1, 256]]))

            # ---- input loads, ordered for earliest stage-1 start: the first
            # token half's transposes land first (B1 tiles 0..7 need only
            # those), rows interleave for the sumsq, the rest trail. ----
            xT = [big.tile([P, N], BF16, tag=f"xT{k}", name=f"xT{k}")
                  for k in range(KT)]
            for k in range(KT):
                nc.sync.dma_start_transpose(out=xT[k][:, 0:HN],
                                            in_=tok_d[0:HN, ts(k, P)])
            for k in range(KT):
                nc.sync.dma_start_transpose(out=xT[k][:, HN:N],
                                            in_=tok_d[HN:N, ts(k, P)])
            wout_sb = const.tile([P, 2, D], BF16)
            nc.sync.dma_start(out=wout_sb, in_=wout_d.rearrange("a p n -> p a n"))

            qk_all = big.tile([P, NT, 384], BF16)
            vext = big.tile([P, NT, 130], BF16)     # [v0 | 1 | v1 | 1]
            nc.gpsimd.memset(vext[:, :, 64:65], 1.0)
            nc.gpsimd.memset(vext[:, :, 129:130], 1.0)
            ssq_all = big.tile([P, NT, 6], F32)
            srq_all = big.tile([P, NT, 6], F32)
            rsq_all = big.tile([P, NT, 6], F32)
            qT_all = big.tile([P, NT, 384], BF16)
            oT_cb = [big.tile([P, N], BF16, tag=f"ocb{ih}", name=f"ocb{ih}")
                     for ih in range(2)]
            oT_nm = [big.tile([P, N], BF16, tag=f"onm{ih}", name=f"onm{ih}")
                     for ih in range(2)]

            # ---------------- emission helpers ----------------
            def emit_b1(t):
                """Project token tile t: q/k to qk_all, raw v to vcp_all,
                plus the token sum-of-squares and head sums-of-squares."""
                pj = pps.tile([P, 512], F32, tag="ps")
                for k in range(KT):
                    nc.tensor.matmul(pj, lhsT=xT[k][:, ts(t, P)],
                                     rhs=wqkv_sb[:, k, :],
                                     start=(k == 0), stop=(k == KT - 1))
                nc.vector.tensor_copy(qk_all[:, t, :], pj[:, 0:384])
                nc.vector.tensor_copy(vext[:, t, 0:64], pj[:, 384:448])
                nc.vector.tensor_copy(vext[:, t, 65:129], pj[:, 448:512])
                sq = work.tile([P, 384], BF16, tag="sq", bufs=1)
                nc.vector.tensor_tensor(out=sq, in0=qk_all[:, t, :],
                                        in1=qk_all[:, t, :], op=ALU.mult)
                nc.vector.tensor_reduce(
                    ssq_all[:, t, :], sq.rearrange("p (h d) -> p h d", d=64),
                    axis=mybir.AxisListType.X, op=ALU.add)
                if t % 2 == 1:
                    emit_group_batch(t - 1)

            def emit_group_batch(g0):
                """Per-2-tile batch: head norms; the k norms carry the 0.02
                softcap prescale: sqrt(2500*ssq) = 50*||k||."""
                nc.scalar.activation(
                    srq_all[:, g0:g0 + 2, 0:4],
                    ssq_all[:, g0:g0 + 2, 0:4], AF.Sqrt)
                nc.scalar.activation(
                    srq_all[:, g0:g0 + 2, 4:6],
                    ssq_all[:, g0:g0 + 2, 4:6], AF.Sqrt, scale=2500.0)
                nc.vector.reciprocal(
                    rsq_all[:, g0:g0 + 2, :].rearrange("p a b -> p (a b)"),
                    srq_all[:, g0:g0 + 2, :].rearrange("p a b -> p (a b)"))

            def emit_b2(t):
                """Scale q heads by 1/||q|| * combined gamma, transpose the
                q blocks + raw k block, assemble qT_all[:, t, :]."""
                qn = work.tile([P, 256], BF16, tag="qn")
                for j in range(4):   # j = 2*ih + g
                    dest = 128 * (j % 2) + 64 * (j // 2)
                    nc.vector.scalar_tensor_tensor(
                        out=qn[:, ds(dest, 64)],
                        in0=qk_all[:, t, ds(64 * j, 64)],
                        scalar=rsq_all[:, t, j:j + 1],
                        in1=gvec_sb[:, ds(64 * j, 64)],
                        op0=ALU.mult, op1=ALU.mult)
                tp = pps.tile([P, 384], BF16, tag="ps")
                nc.tensor.transpose(tp[:, 0:128], qn[:, 0:128], ident)
                nc.tensor.transpose(tp[:, 128:256], qn[:, 128:256], ident)
                nc.tensor.transpose(tp[:, 256:384], qk_all[:, t, 256:384],
                                    ident)
                nc.vector.tensor_copy(qT_all[:, t, :], tp)

            def emit_d_pair(t, c, act_copy, alt_pool=False):
                """One output-projection chunk: out[t-tile, 512c:512c+512]."""
                if alt_pool:
                    op_ps = pst.tile([P, 512], F32, tag="st",
                                     name=f"od{t}{c}")
                else:
                    op_ps = pps.tile([P, 512], F32, tag="ps")
                for jh in range(2):
                    nc.tensor.matmul(op_ps,
                                     lhsT=oT_nm[jh][:, ts(t, P)],
                                     rhs=wout_sb[:, jh, ds(512 * c, 512)],
                                     start=(jh == 0), stop=(jh == 1))
                ob = work.tile([P, 512], BF16, tag="ob")
                if act_copy:
                    nc.scalar.copy(ob, op_ps)
                else:
                    nc.vector.tensor_copy(ob, op_ps)
                nc.sync.dma_start(out=out_d[ts(t, P), ds(512 * c, 512)],
                                  in_=ob)

            og_p = {}
            for ih in range(2):
                for g in range(2):
                    for c in range(4):
                        og_p[(ih, g, c)] = big.tile(
                            [65, 512], BF16, tag=f"og{ih}{g}{c}",
                            name=f"og{ih}{g}{c}")

            def emit_av(otc, jt, i_start, pT, ih, g, hf, njt, on_drain):
                """Accumulate exp(scores) @ [v | 1] into the per-512-chunk
                psum tiles; drain each chunk right after its last key tile."""
                for c in range(2 * hf, 2 * hf + 2):
                    ic = 512 * c
                    if ic + 512 <= i_start:
                        continue
                    off = max(0, i_start - ic)
                    stop_jt = min(njt - 1, 4 * c + 3)
                    nc.tensor.matmul(
                        otc[c][:, ds(off, 512 - off)],
                        lhsT=vext[:, jt, ds(65 * ih, 65)],
                        rhs=pT[:, ds(ic + off - i_start, 512 - off)],
                        start=(jt == 0), stop=(jt == stop_jt))
                    if jt == stop_jt:
                        og = og_p[(ih, g, c)]
                        nc.vector.tensor_copy(og, otc[c])
                        nc.sync.dma_start(
                            out=oT_cb[ih][ds(64 * g, 64), ds(ic, 512)],
                            in_=og[0:64, :])
                        if c in on_drain:
                            on_drain[c]()

            def emit_norm(ih, c):
                """Broadcast the softmax denominators for this 512-chunk
                across the partitions with a ones-matmul, then reciprocal
                and normalize oT (no DRAM roundtrip)."""
                ic = 512 * c
                lbc = pot.tile([P, 512], F32, tag="ot", name=f"lb{ih}{c}")
                for gg in range(2):
                    nc.tensor.matmul(
                        lbc[ds(64 * gg, 64), :],
                        lhsT=ones_t[64:65, :],
                        rhs=og_p[(ih, gg, c)][64:65, :],
                        start=True, stop=True)
                rlb = nrm.tile([P, 512], F32, tag="rlb", bufs=2)
                nc.vector.reciprocal(rlb, lbc)
                nc.vector.tensor_mul(oT_nm[ih][:, ds(ic, 512)],
                                     oT_cb[ih][:, ds(ic, 512)], rlb)

            def emit_inst_half(ih, g, hf, fillers, on_drain=None):
                """One (kv-head, group) instance's hf half of the causal
                attention. `fillers` is a list of thunks (independent work)
                woven between jt iterations to fill other engines; `on_drain`
                maps a chunk index to a thunk run right after its drain."""
                lo, hi = HN * hf, HN * (hf + 1)
                njt = 8 * (hf + 1)
                on_drain = on_drain or {}
                otc = {}
                for c in range(2 * hf, 2 * hf + 2):
                    otc[c] = pot.tile([65, 512], F32, tag="ot",
                                      name=f"ot{ih}{g}{hf}{c}")
                pend = None
                fi = 0
                for jt in range(njt):
                    i_start = max(P * jt, lo)
                    ni = hi - i_start
                    pT = att.tile([P, ni], BF16, tag="pT", bufs=4)
                    st = pst.tile([P, ni], F32, tag="st")
                    for hb in range(0, ni, 512):
                        hw = min(512, ni - hb)
                        t0 = (i_start + hb) // P
                        nc.tensor.matmul(
                            st[:, ds(hb, hw)],
                            lhsT=qT_all[ds(64 * ih, 64), jt, 256:384],
                            rhs=qT_all[ds(64 * ih, 64), t0:t0 + hw // P,
                                       ds(128 * g, 128)],
                            start=True, stop=True)
                    # AV for the PREVIOUS key tile: keeps PE a tile ahead of
                    # ACT so tanh never waits on scores
                    if pend is not None:
                        emit_av(otc, *pend, ih, g, hf, njt, on_drain)
                    if jt % 2 == 0 and fi < len(fillers):
                        fillers[fi]()
                        fi += 1
                    nc.scalar.activation(st, st, AF.Tanh,
                                         scale=rsq_all[:, jt, 4 + ih:5 + ih])
                    nc.scalar.activation(pT, st, AF.Exp, scale=6.25)
                    if i_start == P * jt:
                        # causal mask on the leading diagonal block
                        nc.gpsimd.affine_select(
                            out=pT[:, 0:P], in_=pT[:, 0:P],
                            compare_op=ALU.is_ge, fill=0.0,
                            base=0, pattern=[[1, P]],
                            channel_multiplier=-1)
                    pend = (jt, i_start, pT)
                emit_av(otc, *pend, ih, g, hf, njt, on_drain)
                for f in fillers[fi:]:
                    f()

            # ---------------- stage 1: B for token tiles 0..7; B2 lags two
            # tiles behind B1 ----------------
            for t in range(8):
                emit_b1(t)
                if t % 2 == 1 and t >= 3:
                    emit_b2(t - 3)
                    emit_b2(t - 2)

            # ---------------- stage 2: hf=0 attention (token tiles 0..7 only)
            # interleaved with B for tiles 8..15 ----------------
            for t in range(6, 8):
                emit_b2(t)
            rest = [(lambda tt=t: emit_b1(tt)) for t in range(8, 12)]
            rest += [(lambda tt=t: emit_b2(tt)) for t in range(8, 12)]
            rest += [(lambda tt=t: emit_b1(tt)) for t in range(12, 16)]
            rest += [(lambda tt=t: emit_b2(tt)) for t in range(12, 16)]
            insts = [(0, 0), (0, 1), (1, 0), (1, 1)]
            for idx, (ih, g) in enumerate(insts):
                fillers = rest[4 * idx:4 * idx + 4]
                hooks = None
                if g == 1:
                    hooks = {0: (lambda i_=ih: emit_norm(i_, 0)),
                             1: (lambda i_=ih: emit_norm(i_, 1))}
                emit_inst_half(ih, g, 0, fillers, hooks)

            # ---------------- stage 3: hf=1 attention with the first half of
            # the output projection woven in; the last instance's early chunk
            # drain (c=2 at key tile 11) unlocks out rows 1024:1536 too ----
            d_half0 = [(t, c) for t in range(8) for c in range(2)]

            def last_c2_hook():
                emit_norm(1, 2)
                for t in (8, 9, 10, 11):
                    for c in range(2):
                        emit_d_pair(t, c, act_copy=False)

            for idx, (ih, g) in enumerate(insts):
                fillers = [
                    (lambda tc_=tc2: emit_d_pair(*tc_, act_copy=False))
                    for tc2 in d_half0[4 * idx:4 * idx + 4]]
                hooks = None
                if g == 1:
                    if ih == 0:
                        hooks = {2: lambda: emit_norm(0, 2),
                                 3: lambda: emit_norm(0, 3)}
                    else:
                        hooks = {2: last_c2_hook, 3: lambda: emit_norm(1, 3)}
                emit_inst_half(ih, g, 1, fillers, hooks)

            # ---------------- phase D second half ----------------
            for t in range(12, NT):
                for c in range(2):
                    emit_d_pair(t, c, act_copy=(c == 0), alt_pool=(c == 1))

    if split_waits:
        _split_waits(nc)
    return nc


_NC_CACHE = {}


def _get_nc():
    if "nc" not in _NC_CACHE:
        _NC_CACHE["nc"] = build_nc()
    return _NC_CACHE["nc"]


def _make_in_maps(inputs):
    tokens = np.asarray(inputs["tokens"], np.float32)
    norm_w = np.asarray(inputs["norm_w"], np.float32)
    Wq = np.asarray(inputs["Wq"], np.float32)
    Wkv = np.asarray(inputs["Wkv"], np.float32)
    Wout = np.asarray(inputs["Wout"], np.float32)
    qg = np.asarray(inputs["q_gamma"], np.float32)
    kg = np.asarray(inputs["k_gamma"], np.float32)

    bf = ml_dtypes.bfloat16
    # rmsnorm row scale folded into the tokens (norm_w folds into weights)
    rs = 1.0 / np.sqrt((tokens * tokens).mean(-1, keepdims=True)
                       + np.finfo(np.float32).eps)
    tok_n = tokens * rs
    tok_bf = [tok_n[b].astype(bf) for b in range(B)]
    wq_n = norm_w[:, None] * Wq
    wkv_n = norm_w[:, None] * Wkv

    in_maps = []
    for c in range(8):
        b, hp = c // 4, c % 4
        h0, h1 = 2 * hp, 2 * hp + 1
        qh = 4 * hp
        wqkv = np.concatenate([
            wq_n[:, 64 * qh:64 * (qh + 4)],
            wkv_n[:, 64 * h0:64 * (h1 + 1)],
            wkv_n[:, 512 + 64 * h0:512 + 64 * (h1 + 1)],
        ], axis=1).astype(bf)                                   # [1024, 512]
        wout = np.stack([
            np.concatenate([Wout[64 * h:64 * (h + 1)]] * 2, 0)  # [128, 1024]
            for h in (h0, h1)]).astype(bf)
        # combined q-side gamma: (qg+1)(kg+1)*DH per local q head j = 2*ih+g
        gvec = np.concatenate([
            (qg[qh + j] + 1.0) * (kg[h0 + j // 2] + 1.0) * np.float32(DH)
            for j in range(4)])                                 # [256]
        in_maps.append({
            "tok": np.ascontiguousarray(tok_bf[b]),
            "wqkv": np.ascontiguousarray(wqkv),
            "wout": np.ascontiguousarray(wout),
            "gvec": np.ascontiguousarray(gvec.astype(bf)),
        })
    return in_maps


def _run(inputs, **kw):
    nc = _get_nc()
    in_maps = _make_in_maps(inputs)
    res = run_bass_kernel_spmd(nc, in_maps, core_ids=list(range(8)), **kw)
    out = np.zeros((B, N, D), np.float32)
    for c in range(8):
        out[c // 4] += res.results[c]["out_p"].astype(np.float32)
    return out, res


def kernel(**inputs) -> np.ndarray:
    out, _ = _run(inputs)
    return out


if __name__ == "__main__":
    import reference as R
    inp = {k: np.asarray(v) for k, v in R.setup_inputs().items()}
    exp = np.asarray(R.reference(**inp))
    got = kernel(**inp)
    rel = np.linalg.norm(got - exp) / np.linalg.norm(exp)
    print("Relative error:", rel)
